# revision 1
# baseline (speedup 1.0000x reference)
"""MeshConv-transpose Trainium2 kernel.

out[b,:,n] = (identity @ c0 + L_spmm @ c1 + EW_spmm @ c2 + NS_spmm @ c3 + bias)^T

Strategy (8 NeuronCores): cores = 2 batch-groups x 4 dest-quarters.
- Channel transform FIRST on PE: tables T0 = [x;1] @ c0 and T123 = [x;1] @ c(1..3),
  rows packed [vertex, 4 batches x 64 ch] f32 in HBM scratch.
- Sparse gather: cols >= 10242 hit the ones-padding, so their weighted sum is a
  per-dest scalar computed on host and applied via a tiny [5,128]@[5,256] PE
  matmul (csum rows + bias). Real cols (<10242) become a dma_gather edge list
  (int16 row ids into T123); identity is one more gather slot into T0.
- Per dest-tile (128 dests): gather G[128, S, 256], multiply by per-slot vals
  (broadcast AP), reduce over slots on DVE, add the PE const matmul, DMA out.
- Dests are permuted (degree-sorted, round-robin dealt to quarters) so slot
  counts are uniform across cores; host un-permutes the output.
"""
import numpy as np

import concourse.bass as bass
import concourse.mybir as mybir
import concourse.tile as tile
from concourse import library_config
from concourse.bass_utils import run_bass_kernel_spmd
from concourse.library_overlay import lower_extended_insts

# ---- problem constants (hardcoded per harness contract) ----
NV = 40962
NVPREV = 10242
B = 8
C = 64
K_L = 7
K_G = 18

NBG = 2            # batch groups
BPC = B // NBG     # batches per core (4)
NQ = 4             # dest quarters
NCORES = NBG * NQ

NVQ = 10368        # xq rows per batch (=81*128): 10242 x-cols + ones col + pad
NTILES_V = NVQ // 128   # 81 z-build tiles
NPAD = 4 * NVQ     # padded dest count 41472
DPC = NVQ          # dests per core (10368)
NTILES_D = DPC // 128   # 81 dest tiles
EW = BPC * C       # elem width per table row (256 f32)

f32 = mybir.dt.float32
bf16 = mybir.dt.bfloat16
i16 = mybir.dt.int16
NP_BF16 = mybir.dt.np(bf16)


def _fix_multiwait(nc, max_waits=1):
    """This walrus build accepts one sem-wait per instruction; hoist extras
    onto same-engine no-ops spliced before the instruction."""
    for f in nc.m.functions:
        for bb in f.blocks:
            out, changed = [], False
            for inst in bb.instructions:
                si = inst.sync_info
                waits = list(si.on_wait) if si and si.on_wait else []
                if len(waits) > max_waits:
                    for w in waits[:-max_waits]:
                        nop = mybir.InstNoOp(
                            name=nc.get_next_instruction_name(),
                            engine=inst.engine, ins=[], outs=[],
                            sync_info=mybir.SyncInfo(on_wait=[w], on_update=[]),
                        )
                        nc.register_instruction(nop)
                        out.append(nop)
                    si.on_wait = waits[-max_waits:]
                    changed = True
                out.append(inst)
            if changed:
                bb.instructions = out


def _wrap_idx(idx_flat):
    """Pack a flat index list (len = multiple of 128) into the dma_gather idx
    tile layout: wrapped into 16 partitions, replicated to 8 Q7 cores."""
    n = len(idx_flat)
    w = np.zeros((16, n // 16), np.int16)
    q = np.arange(n)
    w[q % 16, q // 16] = idx_flat
    return np.tile(w, (8, 1))  # [128, n//16]


def _preprocess(x, L_cols, L_vals, EW_cols, EW_vals, NS_cols, NS_vals, coeffs, bias):
    """Host-side packing. Returns per-quarter gather metadata + shared consts."""
    cols_ops = [np.asarray(L_cols), np.asarray(EW_cols), np.asarray(NS_cols)]
    vals_ops = [np.asarray(L_vals), np.asarray(EW_vals), np.asarray(NS_vals)]

    # per-dest real edges (col < NVPREV) and pad-sums, over the 3 ops
    real_masks = [c < NVPREV for c in cols_ops]
    deg_ops = [m.sum(1) for m in real_masks]           # [NV] per op
    deg = sum(deg_ops)                                 # total real degree per dest
    s_pad = [np.where(~m, v, 0).sum(1).astype(np.float32)
             for m, v in zip(real_masks, vals_ops)]    # [NV] per op

    # permutation: degree-sorted (desc), padded dests at the end, dealt to quarters
    deg_pad = np.concatenate([deg, np.full(NPAD - NV, -1)])
    order = np.argsort(-deg_pad, kind="stable")
    pis = [order[q::NQ] for q in range(NQ)]            # [DPC] original dest ids

    # per-tile slot count, shared across quarters (program must be uniform)
    deg_of = lambda pi: np.where(pi < NV, deg_pad[np.minimum(pi, NV - 1)], 0).clip(0)
    S_t = np.zeros(NTILES_D, np.int64)
    for t in range(NTILES_D):
        m = 0
        for q in range(NQ):
            pi = pis[q][t * 128:(t + 1) * 128]
            m = max(m, int(deg_of(pi).max()))
        S_t[t] = 1 + m

    # pack per-quarter gather data
    quarters = []
    for q in range(NQ):
        pi = pis[q]
        idx0 = np.zeros((NTILES_D, 128, 8), np.int16)
        idx123_cols, vals_cols = [], []
        s5 = np.zeros((NTILES_D, 5, 128), np.float32)
        for t in range(NTILES_D):
            p_ids = pi[t * 128:(t + 1) * 128]
            real = p_ids < NV
            st = int(S_t[t])
            # identity slot
            id_idx = np.where(p_ids < NVPREV, p_ids, NVPREV).astype(np.int16)
            idx0[t] = _wrap_idx(np.where(real, id_idx, 0))
            # edge slots
            eidx = np.zeros((st - 1, 128), np.int16)
            eval_ = np.zeros((128, st), np.float32)
            eval_[:, 0] = real.astype(np.float32)
            for p in range(128):
                n = p_ids[p]
                if n >= NV:
                    continue
                d = 0
                for k in range(3):
                    cs_, vs_ = cols_ops[k][n], vals_ops[k][n]
                    m = real_masks[k][n]
                    cc, vv = cs_[m], vs_[m]
                    eidx[d:d + len(cc), p] = (k * NVQ + cc).astype(np.int16)
                    eval_[p, 1 + d:1 + d + len(vv)] = vv
                    d += len(cc)
                s5[t, 1, p] = s_pad[0][n]
                s5[t, 2, p] = s_pad[1][n]
                s5[t, 3, p] = s_pad[2][n]
                s5[t, 4, p] = 1.0
            idx123_cols.append(_wrap_idx(eidx.reshape(-1)))
            vals_cols.append(eval_)
        quarters.append(dict(
            pi=pi,
            idx0=idx0,
            idx123=np.concatenate(idx123_cols, axis=1),
            vals=np.concatenate(vals_cols, axis=1),
            s5=s5,
        ))

    # shared consts
    coeffs = np.asarray(coeffs, np.float32)
    callw = np.concatenate([coeffs[k] for k in range(4)], axis=1).astype(NP_BF16)
    csum = coeffs.sum(axis=1)                                      # [4, 64]
    cs = np.zeros((5, EW), np.float32)
    for k in range(1, 4):
        cs[k] = np.tile(csum[k], BPC)
    cs[4] = np.tile(np.asarray(bias, np.float32), BPC)

    # xq per batch group: [BPC, 64, NVQ], col NVPREV = ones
    x = np.asarray(x, np.float32)
    xqs = []
    for g in range(NBG):
        xq = np.zeros((BPC, C, NVQ), np.float32)
        xq[:, :, :NVPREV] = x[g * BPC:(g + 1) * BPC]
        xq[:, :, NVPREV] = 1.0
        xqs.append(xq.astype(NP_BF16))

    return quarters, xqs, callw, cs, S_t


def _build_program(S_t, wtot, stot, phase1=True, phase2=True, repeat=1,
                   n_queues=1, single_packet=True):
    nc = bass.Bass(num_swdge_queues=n_queues)
    xq_ext = nc.declare_dram_parameter("xq", [BPC, C, NVQ], bf16, isOutput=False)
    callw_ext = nc.declare_dram_parameter("callw", [C, 4 * C], bf16, isOutput=False)
    cs_ext = nc.declare_dram_parameter("cs", [5, EW], f32, isOutput=False)
    idx0_ext = nc.declare_dram_parameter("idx0", [NTILES_D, 128, 8], i16, isOutput=False)
    idx123_ext = nc.declare_dram_parameter("idx123", [128, wtot], i16, isOutput=False)
    vals_ext = nc.declare_dram_parameter("vals", [128, stot], f32, isOutput=False)
    s5_ext = nc.declare_dram_parameter("s5", [NTILES_D, 5, 128], f32, isOutput=False)
    out_ext = nc.declare_dram_parameter("out", [DPC, EW], f32, isOutput=True)

    t0_dram = nc.dram_tensor("t0_scratch", [NVQ, EW], bf16)
    t123_dram = nc.dram_tensor("t123_scratch", [3 * NVQ, EW], bf16)

    s_max = int(S_t.max())

    with tile.TileContext(nc) as tc:
        with (
            tc.tile_pool(name="const", bufs=1) as constp,
            tc.tile_pool(name="psum", bufs=4, space="PSUM") as psum,
        ):
            nc.gpsimd.load_library(library_config.mlp)
            callw_t = constp.tile([C, 4 * C], bf16)
            cs_t = constp.tile([5, EW], f32)
            nc.sync.dma_start(callw_t[:], callw_ext[:])
            nc.sync.dma_start(cs_t[:], cs_ext[:])

            gq = [0]  # round-robin gather queue counter

            def _body():
                if phase1:
                    _phase1()
                if phase2:
                    _phase2()

            def _phase1():
                with (
                    tc.tile_pool(name="xq", bufs=1) as xqp,
                    tc.tile_pool(name="zstage", bufs=3) as zst,
                ):
                    xq_t = [xqp.tile([C, NVQ], bf16, tag=f"xq{bb}", name=f"xq{bb}")
                            for bb in range(BPC)]
                    for bb in range(BPC):
                        nc.sync.dma_start(xq_t[bb][:], xq_ext[bb])
                    for vt in range(NTILES_V):
                        sl = slice(vt * 128, (vt + 1) * 128)
                        st0 = zst.tile([128, EW], bf16, tag="st0")
                        st123 = zst.tile([128, 3, EW], bf16, tag="st123")
                        for bb in range(BPC):
                            ps = psum.tile([128, 4 * C], f32, tag="zps")
                            nc.tensor.matmul(ps[:], xq_t[bb][:, sl], callw_t[:],
                                             start=True, stop=True)
                            if bb % 2 == 0:
                                ceng = nc.scalar.copy
                            else:
                                ceng = nc.vector.tensor_copy
                            ceng(st0[:, bb * C:(bb + 1) * C], ps[:, 0:C])
                            # ops 1..3 -> st123[:, k, bb*C:(bb+1)*C]
                            ceng(
                                st123[:, :, bb * C:(bb + 1) * C],
                                ps[:, C:4 * C].rearrange("p (k c) -> p k c", k=3),
                            )
                        nc.sync.dma_start(t0_dram[sl], st0[:])
                        for k in range(3):
                            nc.sync.dma_start(t123_dram[k * NVQ + vt * 128:
                                                        k * NVQ + vt * 128 + 128],
                                              st123[:, k, :])

            reg_cache = {}

            def nreg(v):
                if v not in reg_cache:
                    reg_cache[v] = nc.gpsimd.to_reg(v)
                return reg_cache[v]

            def _gather(out_ap, tab, idxs, n):
                q = gq[0] % n_queues
                gq[0] += 1
                nc.gpsimd.dma_gather(out_ap, tab, idxs, num_idxs=n,
                                     num_idxs_reg=nreg(n), elem_size=EW,
                                     queue_num=q, single_packet=single_packet)

            def _phase2():
                do_gather = phase2 in (True, "gather")
                do_compute = phase2 in (True, "compute")
                GRP = 8
                with tc.tile_pool(name="work", bufs=3) as work:
                    woff = 0
                    voff = 0
                    gw = gv = 0
                    for t in range(NTILES_D):
                        st = int(S_t[t])
                        wt = (st - 1) * 8
                        if t % GRP == 0:
                            tn = min(GRP, NTILES_D - t)
                            gwid = sum((int(S_t[u]) - 1) * 8 for u in range(t, t + tn))
                            gsl = sum(int(S_t[u]) for u in range(t, t + tn))
                            idx0_g = work.tile([128, GRP, 8], i16, tag="idx0")
                            idx123_g = work.tile([128, max(gwid, 1)], i16, tag="idx123")
                            vals_g = work.tile([128, gsl], f32, tag="vals")
                            s5_g = work.tile([5, GRP, 128], f32, tag="s5")
                            nc.sync.dma_start(
                                idx0_g[:, :tn, :],
                                idx0_ext[t:t + tn].transpose([1, 0, 2]))
                            if gwid:
                                nc.sync.dma_start(idx123_g[:, :gwid],
                                                  idx123_ext[:, woff:woff + gwid])
                            nc.sync.dma_start(vals_g[:, :gsl],
                                              vals_ext[:, voff:voff + gsl])
                            nc.sync.dma_start(
                                s5_g[:, :tn, :],
                                s5_ext[t:t + tn].transpose([1, 0, 2]))
                            gw = gv = 0
                        ti = t % GRP
                        idx0_t = idx0_g[:, ti, :]
                        idx123_t = idx123_g[:, gw:gw + max(wt, 1)]
                        vals_t = vals_g[:, gv:gv + st]
                        s5_t = s5_g[:, ti, :]

                        G = work.tile([128, s_max, EW], bf16, tag="G")
                        if do_gather:
                            _gather(G[:, 0:1, :], t0_dram[:], idx0_t, 128)
                            # descriptor ring fits 1024 descs per call
                            d0 = 1
                            while d0 < st:
                                dn = min(st - d0, 8)
                                c0 = (d0 - 1) * 8
                                _gather(G[:, d0:d0 + dn, :], t123_dram[:],
                                        idx123_g[:, gw + c0:gw + c0 + dn * 8],
                                        dn * 128)
                                d0 += dn
                        if not do_compute:
                            nc.sync.dma_start(out_ext[t * 128:(t + 1) * 128],
                                              G[:, 0, :])
                            woff += wt
                            voff += st
                            continue
                        scaled = work.tile([128, s_max, EW], bf16, tag="scaled")
                        for d in range(st):
                            nc.vector.tensor_scalar_mul(scaled[:, d, :], G[:, d, :],
                                                        vals_g[:, gv + d:gv + d + 1])
                        # tree-fold slots in bf16 (TT 2x), final reduce on <=4
                        cur = st
                        while cur > 4:
                            lo = cur // 2
                            hi = cur - lo
                            nc.vector.tensor_tensor(
                                out=scaled[:, :lo, :], in0=scaled[:, :lo, :],
                                in1=scaled[:, hi:cur, :], op=mybir.AluOpType.add)
                            cur = hi
                        acc = work.tile([128, EW], f32, tag="acc")
                        nc.vector.tensor_reduce(acc[:],
                                                scaled[:, :cur, :].transpose([0, 2, 1]),
                                                axis=mybir.AxisListType.X,
                                                op=mybir.AluOpType.add)
                        ps2 = psum.tile([128, EW], f32, tag="cps")
                        nc.tensor.matmul(ps2[:], s5_t, cs_t[:], start=True, stop=True)
                        outt = work.tile([128, EW], f32, tag="outt")
                        nc.vector.tensor_tensor(out=outt[:], in0=acc[:], in1=ps2[:],
                                                op=mybir.AluOpType.add)
                        nc.sync.dma_start(out_ext[t * 128:(t + 1) * 128], outt[:])
                        woff += wt
                        voff += st
                        gw += wt
                        gv += st

            for _ in range(repeat):
                _body()

    lower_extended_insts(nc)
    _fix_multiwait(nc)
    return nc


def kernel(x, L_cols, L_vals, EW_cols, EW_vals, NS_cols, NS_vals, coeffs, bias):
    quarters, xqs, callw, cs, S_t = _preprocess(
        x, L_cols, L_vals, EW_cols, EW_vals, NS_cols, NS_vals, coeffs, bias)

    wtot = quarters[0]["idx123"].shape[1]
    stot = quarters[0]["vals"].shape[1]
    # quarters share S_t so widths match by construction
    assert all(qd["idx123"].shape[1] == wtot for qd in quarters)

    nc = _build_program(S_t, wtot, stot)

    in_maps = []
    for c in range(NCORES):
        g, q = divmod(c, NQ)
        qd = quarters[q]
        in_maps.append({
            "xq": xqs[g],
            "callw": callw,
            "cs": cs,
            "idx0": qd["idx0"],
            "idx123": qd["idx123"],
            "vals": qd["vals"],
            "s5": qd["s5"],
        })

    res = run_bass_kernel_spmd(nc, in_maps, list(range(NCORES)))

    out = np.zeros((B, C, NV), np.float32)
    for c in range(NCORES):
        g, q = divmod(c, NQ)
        pi = quarters[q]["pi"]
        o = res.results[c]["out"]              # [DPC, EW]
        valid = pi < NV
        rows = o[valid].reshape(-1, BPC, C)    # [nvalid, b, c]
        out[g * BPC:(g + 1) * BPC, :, pi[valid]] = rows.transpose(1, 2, 0)
    return out



# revision 5
# speedup vs baseline: 1.2691x; 1.2691x over previous
"""MeshConv-transpose Trainium2 kernel.

out[b,:,n] = (identity @ c0 + L_spmm @ c1 + EW_spmm @ c2 + NS_spmm @ c3 + bias)^T

Strategy (8 NeuronCores): cores = 2 batch-groups x 4 dest-quarters.
- Phase 1 (tables): T0 = [x;1] @ c0 and T123 = [x;1] @ c(1..3) on PE,
  rows packed [vertex, 4 batches x 64 ch] bf16 in HBM scratch.
- Phase 2 (sparse): per dest-tile (128 dests), gather identity + edge rows
  (dma_gather, 512B descriptors), scale each slot on DVE (tensor_scalar,
  4x bf16 mode), and accumulate slots on the PE via identity-matmuls into
  PSUM (out[m,n] = sum_k eye[k,m]*slot[k,n]); the per-dest pad-sum/bias
  const matmul (s5 @ cs) accumulates into the same PSUM bank. ACT copies
  PSUM -> bf16 out tile; host casts back to f32.
- Gathers are batched: tiles are packed into groups (<= GMAX slots); edge
  descriptors stream contiguously across tiles so one call covers many
  tiles. Identity descs batched per group. Big SWDGE ring, 2 queues.
- Dests are permuted (degree-sorted, round-robin dealt to quarters) so slot
  counts are uniform across cores; host un-permutes the output.
"""
import numpy as np

import concourse.bass as bass
import concourse.mybir as mybir
import concourse.tile as tile
from concourse import library_config
from concourse.bass_utils import run_bass_kernel_spmd
from concourse.library_overlay import lower_extended_insts

# ---- problem constants (hardcoded per harness contract) ----
NV = 40962
NVPREV = 10242
B = 8
C = 64
K_L = 7
K_G = 18

NBG = 2            # batch groups
BPC = B // NBG     # batches per core (4)
NQ = 4             # dest quarters
NCORES = NBG * NQ

NVQ = 10368        # xq rows per batch (=81*128): 10242 x-cols + ones col + pad
NTILES_V = NVQ // 128   # 81 z-build tiles
NPAD = 4 * NVQ     # padded dest count 41472
DPC = NVQ          # dests per core (10368)
NTILES_D = DPC // 128   # 81 dest tiles
EW = BPC * C       # elem width per table row (256 f32)

import os
GMAX = 96          # max G-buffer slots per tile group
GMAX_T = 8         # max tiles per group (identity gather <= 1024 descs)
CHUNK_SLOTS = int(os.environ.get("MC_CHUNK", "16"))   # edge-gather slots/call
SCRATCH = int(os.environ.get("MC_SCRATCH", "65536"))  # SWDGE ring bytes
NQUEUES = int(os.environ.get("MC_NQ", "2"))

f32 = mybir.dt.float32
bf16 = mybir.dt.bfloat16
i16 = mybir.dt.int16
NP_BF16 = mybir.dt.np(bf16)


def _fix_multiwait(nc, max_waits=1):
    """This walrus build accepts one sem-wait per instruction; hoist extras
    onto same-engine no-ops spliced before the instruction."""
    for f in nc.m.functions:
        for bb in f.blocks:
            out, changed = [], False
            for inst in bb.instructions:
                si = inst.sync_info
                waits = list(si.on_wait) if si and si.on_wait else []
                if len(waits) > max_waits:
                    for w in waits[:-max_waits]:
                        nop = mybir.InstNoOp(
                            name=nc.get_next_instruction_name(),
                            engine=inst.engine, ins=[], outs=[],
                            sync_info=mybir.SyncInfo(on_wait=[w], on_update=[]),
                        )
                        nc.register_instruction(nop)
                        out.append(nop)
                    si.on_wait = waits[-max_waits:]
                    changed = True
                out.append(inst)
            if changed:
                bb.instructions = out


def _wrap_idx(idx_flat):
    """Pack a flat index list (len = multiple of 128) into the dma_gather idx
    tile layout: wrapped into 16 partitions, replicated to 8 Q7 cores."""
    n = len(idx_flat)
    w = np.zeros((16, n // 16), np.int16)
    q = np.arange(n)
    w[q % 16, q // 16] = idx_flat
    return np.tile(w, (8, 1))  # [128, n//16]


def _build_groups(S_t):
    """Pack consecutive dest tiles into groups with <= GMAX G-buffer slots
    (1 identity slot + S_t-1 edge slots per tile) and <= GMAX_T tiles."""
    groups = []
    ta = 0
    while ta < NTILES_D:
        nt, slots = 0, 0
        while ta + nt < NTILES_D and nt < GMAX_T:
            add = int(S_t[ta + nt])  # 1 identity + (S_t-1) edges
            if slots + add > GMAX and nt > 0:
                break
            slots += add
            nt += 1
        groups.append((ta, ta + nt))
        ta += nt
    return groups


def _preprocess(x, L_cols, L_vals, EW_cols, EW_vals, NS_cols, NS_vals, coeffs, bias):
    """Host-side packing. Returns per-quarter gather metadata + shared consts."""
    cols_ops = [np.asarray(L_cols), np.asarray(EW_cols), np.asarray(NS_cols)]
    vals_ops = [np.asarray(L_vals), np.asarray(EW_vals), np.asarray(NS_vals)]

    # per-dest real edges (col < NVPREV) and pad-sums, over the 3 ops
    real_masks = [c < NVPREV for c in cols_ops]
    deg_ops = [m.sum(1) for m in real_masks]           # [NV] per op
    deg = sum(deg_ops)                                 # total real degree per dest
    s_pad = [np.where(~m, v, 0).sum(1).astype(np.float32)
             for m, v in zip(real_masks, vals_ops)]    # [NV] per op

    # permutation: degree-sorted (desc), padded dests at the end, dealt to quarters
    deg_pad = np.concatenate([deg, np.full(NPAD - NV, -1)])
    order = np.argsort(-deg_pad, kind="stable")
    pis = [order[q::NQ] for q in range(NQ)]            # [DPC] original dest ids

    # per-tile slot count, shared across quarters (program must be uniform)
    deg_of = lambda pi: np.where(pi < NV, deg_pad[np.minimum(pi, NV - 1)], 0).clip(0)
    S_t = np.zeros(NTILES_D, np.int64)
    for t in range(NTILES_D):
        m = 0
        for q in range(NQ):
            pi = pis[q][t * 128:(t + 1) * 128]
            m = max(m, int(deg_of(pi).max()))
        S_t[t] = 1 + m

    # pack per-quarter gather data
    quarters = []
    for q in range(NQ):
        pi = pis[q]
        idx0_cols, idx123_cols, vals_cols = [], [], []
        s5 = np.zeros((NTILES_D, 5, 128), np.float32)
        for t in range(NTILES_D):
            p_ids = pi[t * 128:(t + 1) * 128]
            real = p_ids < NV
            st = int(S_t[t])
            # identity slot
            id_idx = np.where(p_ids < NVPREV, p_ids, NVPREV).astype(np.int16)
            idx0_cols.append(_wrap_idx(np.where(real, id_idx, 0)))
            # edge slots
            eidx = np.zeros((st - 1, 128), np.int16)
            eval_ = np.zeros((128, st), np.float32)
            eval_[:, 0] = real.astype(np.float32)
            for p in range(128):
                n = p_ids[p]
                if n >= NV:
                    continue
                d = 0
                for k in range(3):
                    cs_, vs_ = cols_ops[k][n], vals_ops[k][n]
                    m = real_masks[k][n]
                    cc, vv = cs_[m], vs_[m]
                    eidx[d:d + len(cc), p] = (k * NVQ + cc).astype(np.int16)
                    eval_[p, 1 + d:1 + d + len(vv)] = vv
                    d += len(cc)
                s5[t, 1, p] = s_pad[0][n]
                s5[t, 2, p] = s_pad[1][n]
                s5[t, 3, p] = s_pad[2][n]
                s5[t, 4, p] = 1.0
            idx123_cols.append(_wrap_idx(eidx.reshape(-1)))
            vals_cols.append(eval_)
        quarters.append(dict(
            pi=pi,
            idx0=np.concatenate(idx0_cols, axis=1),      # [128, NTILES_D*8]
            idx123=np.concatenate(idx123_cols, axis=1),  # [128, wtot]
            vals=np.concatenate(vals_cols, axis=1),      # [128, stot]
            s5=s5.transpose(1, 0, 2).reshape(5, -1).astype(NP_BF16),
        ))

    # shared consts
    coeffs = np.asarray(coeffs, np.float32)
    callw = np.concatenate([coeffs[k] for k in range(4)], axis=1).astype(NP_BF16)
    csum = coeffs.sum(axis=1)                                      # [4, 64]
    cs = np.zeros((5, EW), np.float32)
    for k in range(1, 4):
        cs[k] = np.tile(csum[k], BPC)
    cs[4] = np.tile(np.asarray(bias, np.float32), BPC)
    cs = cs.astype(NP_BF16)
    eye = np.eye(128, dtype=NP_BF16)

    # xq per batch group: [BPC, 64, NVQ], col NVPREV = ones
    x = np.asarray(x, np.float32)
    xqs = []
    for g in range(NBG):
        xq = np.zeros((BPC, C, NVQ), np.float32)
        xq[:, :, :NVPREV] = x[g * BPC:(g + 1) * BPC]
        xq[:, :, NVPREV] = 1.0
        xqs.append(xq.astype(NP_BF16))

    return quarters, xqs, callw, cs, eye, S_t


def _build_program(S_t, wtot, stot, phase1=True, phase2=True,
                   n_queues=NQUEUES, single_packet=True):
    nc = bass.Bass(num_swdge_queues=n_queues,
                   dynamic_dma_scratch_size=SCRATCH)
    xq_ext = nc.declare_dram_parameter("xq", [BPC, C, NVQ], bf16, isOutput=False)
    callw_ext = nc.declare_dram_parameter("callw", [C, 4 * C], bf16, isOutput=False)
    cs_ext = nc.declare_dram_parameter("cs", [5, EW], bf16, isOutput=False)
    eye_ext = nc.declare_dram_parameter("eye", [128, 128], bf16, isOutput=False)
    idx0_ext = nc.declare_dram_parameter("idx0", [128, NTILES_D * 8], i16, isOutput=False)
    idx123_ext = nc.declare_dram_parameter("idx123", [128, wtot], i16, isOutput=False)
    vals_ext = nc.declare_dram_parameter("vals", [128, stot], f32, isOutput=False)
    s5_ext = nc.declare_dram_parameter("s5", [5, NTILES_D * 128], bf16, isOutput=False)
    out_ext = nc.declare_dram_parameter("out", [DPC, EW], bf16, isOutput=True)

    t0_dram = nc.dram_tensor("t0_scratch", [NVQ, EW], bf16)
    t123_dram = nc.dram_tensor("t123_scratch", [3 * NVQ, EW], bf16)

    groups = _build_groups(S_t)

    with tile.TileContext(nc) as tc:
        with (
            tc.tile_pool(name="const", bufs=1) as constp,
            tc.tile_pool(name="psum", bufs=4, space="PSUM") as psum,
        ):
            nc.gpsimd.load_library(library_config.mlp)
            callw_t = constp.tile([C, 4 * C], bf16)
            cs_t = constp.tile([5, EW], bf16)
            eye_t = constp.tile([128, 128], bf16)
            idx0_t = constp.tile([128, NTILES_D * 8], i16)
            idx123_t = constp.tile([128, wtot], i16)
            vals_t = constp.tile([128, stot], f32)
            s5_t = constp.tile([5, NTILES_D * 128], bf16)
            nc.sync.dma_start(callw_t[:], callw_ext[:])
            nc.sync.dma_start(cs_t[:], cs_ext[:])
            nc.sync.dma_start(eye_t[:], eye_ext[:])
            nc.sync.dma_start(idx0_t[:], idx0_ext[:])
            nc.sync.dma_start(idx123_t[:], idx123_ext[:])
            nc.sync.dma_start(vals_t[:], vals_ext[:])
            nc.sync.dma_start(s5_t[:], s5_ext[:])

            gq = [0]  # round-robin gather queue counter

            def _phase1():
                with (
                    tc.tile_pool(name="xq", bufs=1) as xqp,
                    tc.tile_pool(name="zstage", bufs=3) as zst,
                ):
                    xq_t = [xqp.tile([C, NVQ], bf16, tag=f"xq{bb}", name=f"xq{bb}")
                            for bb in range(BPC)]
                    for bb in range(BPC):
                        nc.sync.dma_start(xq_t[bb][:], xq_ext[bb])
                    t123_3d = t123_dram.rearrange("(k n) e -> k n e", k=3)
                    for vt in range(NTILES_V):
                        sl = slice(vt * 128, (vt + 1) * 128)
                        stall = zst.tile([128, 4, EW], bf16, tag="stall")
                        for bb in range(BPC):
                            ps = psum.tile([128, 4 * C], f32, tag="zps")
                            nc.tensor.matmul(ps[:], xq_t[bb][:, sl], callw_t[:],
                                             start=True, stop=True)
                            if bb % 2 == 0:
                                ceng = nc.scalar.copy
                            else:
                                ceng = nc.vector.tensor_copy
                            ceng(
                                stall[:, :, bb * C:(bb + 1) * C],
                                ps[:, :].rearrange("p (k c) -> p k c", k=4),
                            )
                        nc.sync.dma_start(t0_dram[sl], stall[:, 0, :])
                        for k in range(3):
                            nc.sync.dma_start(t123_3d[k, sl, :],
                                              stall[:, 1 + k, :])

            reg_cache = {}

            def nreg(v):
                if v not in reg_cache:
                    reg_cache[v] = nc.gpsimd.to_reg(v)
                return reg_cache[v]

            def _gather(out_ap, tab, idxs, n):
                q = gq[0] % n_queues
                gq[0] += 1
                nc.gpsimd.dma_gather(out_ap, tab, idxs, num_idxs=n,
                                     num_idxs_reg=nreg(n), elem_size=EW,
                                     queue_num=q, single_packet=single_packet)

            def _phase2():
                with (
                    tc.tile_pool(name="work", bufs=2) as work,
                    tc.tile_pool(name="scaled", bufs=4) as scp,
                    tc.tile_pool(name="outp", bufs=3) as outp,
                ):
                    woff = 0   # global edge-slot offset (idx123 cols = 8/slot)
                    voff = 0   # global vals col offset
                    for (ta, tb) in groups:
                        nt = tb - ta
                        eslots = sum(int(S_t[u]) - 1 for u in range(ta, tb))
                        G = work.tile([128, GMAX, EW], bf16, tag="G")
                        if phase2 in (True, "gather", "compute"):
                            # identity rows for nt tiles in one call
                            _gather(G[:, 0:nt, :], t0_dram[:],
                                    idx0_t[:, ta * 8:tb * 8], nt * 128)
                            # edge rows: contiguous desc stream across tiles
                            e0 = 0
                            while e0 < eslots:
                                en = min(eslots - e0, CHUNK_SLOTS)
                                _gather(G[:, nt + e0:nt + e0 + en, :],
                                        t123_dram[:],
                                        idx123_t[:, (woff + e0) * 8:
                                                 (woff + e0 + en) * 8],
                                        en * 128)
                                e0 += en
                        eoff = nt
                        gv = voff
                        for ti in range(nt):
                            t = ta + ti
                            st = int(S_t[t])
                            acc = psum.tile([128, EW], f32, tag="acc")
                            nc.tensor.matmul(acc[:],
                                             s5_t[:, t * 128:(t + 1) * 128],
                                             cs_t[:], start=True, stop=False)
                            slots = [ti] + list(range(eoff, eoff + st - 1))
                            for j, scol in enumerate(slots):
                                sc = scp.tile([128, EW], bf16, tag="sc")
                                nc.vector.tensor_scalar_mul(
                                    sc[:], G[:, scol, :],
                                    vals_t[:, gv + j:gv + j + 1])
                                nc.tensor.matmul(acc[:], eye_t[:], sc[:],
                                                 start=False,
                                                 stop=(j == st - 1))
                            outt = outp.tile([128, EW], bf16, tag="outt")
                            nc.scalar.copy(outt[:], acc[:])
                            nc.sync.dma_start(out_ext[t * 128:(t + 1) * 128],
                                              outt[:])
                            eoff += st - 1
                            gv += st
                        woff += eslots
                        voff = gv

            if phase1:
                _phase1()
            if phase2:
                _phase2()

    lower_extended_insts(nc)
    _fix_multiwait(nc)
    return nc


def kernel(x, L_cols, L_vals, EW_cols, EW_vals, NS_cols, NS_vals, coeffs, bias):
    quarters, xqs, callw, cs, eye, S_t = _preprocess(
        x, L_cols, L_vals, EW_cols, EW_vals, NS_cols, NS_vals, coeffs, bias)

    wtot = quarters[0]["idx123"].shape[1]
    stot = quarters[0]["vals"].shape[1]
    # quarters share S_t so widths match by construction
    assert all(qd["idx123"].shape[1] == wtot for qd in quarters)

    nc = _build_program(S_t, wtot, stot)

    in_maps = []
    for c in range(NCORES):
        g, q = divmod(c, NQ)
        qd = quarters[q]
        in_maps.append({
            "xq": xqs[g],
            "callw": callw,
            "cs": cs,
            "eye": eye,
            "idx0": qd["idx0"],
            "idx123": qd["idx123"],
            "vals": qd["vals"],
            "s5": qd["s5"],
        })

    res = run_bass_kernel_spmd(nc, in_maps, list(range(NCORES)))

    out = np.zeros((B, C, NV), np.float32)
    for c in range(NCORES):
        g, q = divmod(c, NQ)
        pi = quarters[q]["pi"]
        o = np.asarray(res.results[c]["out"]).astype(np.float32)  # [DPC, EW]
        valid = pi < NV
        rows = o[valid].reshape(-1, BPC, C)    # [nvalid, b, c]
        out[g * BPC:(g + 1) * BPC, :, pi[valid]] = rows.transpose(1, 2, 0)
    return out


# revision 11
# speedup vs baseline: 1.6117x; 1.2700x over previous
"""MeshConv-transpose Trainium2 kernel.

out[b,:,n] = (identity @ c0 + L_spmm @ c1 + EW_spmm @ c2 + NS_spmm @ c3 + bias)^T

Strategy (8 NeuronCores): cores = 2 batch-groups x 4 dest-quarters.
- Phase 1 (tables): T0 = [x;1] @ c0 and T123 = [x;1] @ c(1..3) on PE,
  rows packed [vertex, 4 batches x 64 ch] bf16 in HBM scratch.
- Phase 2 (sparse): per dest-tile (128 dests), gather identity + edge rows
  (dma_gather, 512B descriptors), scale each slot on DVE (tensor_scalar,
  4x bf16 mode), and accumulate slots on the PE via identity-matmuls into
  PSUM (out[m,n] = sum_k eye[k,m]*slot[k,n]); the per-dest pad-sum/bias
  const matmul (s5 @ cs) accumulates into the same PSUM bank. ACT copies
  PSUM -> bf16 out tile; host casts back to f32.
- Gathers are batched: tiles are packed into groups (<= GMAX slots); edge
  descriptors stream contiguously across tiles so one call covers many
  tiles. Identity descs batched per group. Big SWDGE ring, 2 queues.
- Dests are permuted (degree-sorted, round-robin dealt to quarters) so slot
  counts are uniform across cores; host un-permutes the output.
"""
import numpy as np

import concourse.bass as bass
import concourse.mybir as mybir
import concourse.tile as tile
from concourse import library_config
from concourse.bass_utils import run_bass_kernel_spmd
from concourse.library_overlay import lower_extended_insts

# ---- problem constants (hardcoded per harness contract) ----
NV = 40962
NVPREV = 10242
B = 8
C = 64
K_L = 7
K_G = 18

NBG = 2            # batch groups
BPC = B // NBG     # batches per core (4)
NQ = 4             # dest quarters
NCORES = NBG * NQ

NVQ = 10368        # xq rows per batch (=81*128): 10242 x-cols + ones col + pad
NTILES_V = NVQ // 128   # 81 z-build tiles
NPAD = 4 * NVQ     # padded dest count 41472
DPC = NVQ          # dests per core (10368)
NTILES_D = DPC // 128   # 81 dest tiles
EW = BPC * C       # elem width per table row (256 f32)

import os
GMAX = 96          # max G-buffer slots per tile group
GMAX_T = 8         # max tiles per group (identity gather <= 1024 descs)
CHUNK_SLOTS = int(os.environ.get("MC_CHUNK", "8"))    # edge-gather slots/call
SCRATCH = int(os.environ.get("MC_SCRATCH", "16384"))  # SWDGE ring bytes
NQUEUES = int(os.environ.get("MC_NQ", "2"))

f32 = mybir.dt.float32
bf16 = mybir.dt.bfloat16
i16 = mybir.dt.int16
NP_BF16 = mybir.dt.np(bf16)


def _fix_multiwait(nc, max_waits=1):
    """This walrus build accepts one sem-wait per instruction; hoist extras
    onto same-engine no-ops spliced before the instruction."""
    for f in nc.m.functions:
        for bb in f.blocks:
            out, changed = [], False
            for inst in bb.instructions:
                si = inst.sync_info
                waits = list(si.on_wait) if si and si.on_wait else []
                if len(waits) > max_waits:
                    for w in waits[:-max_waits]:
                        nop = mybir.InstNoOp(
                            name=nc.get_next_instruction_name(),
                            engine=inst.engine, ins=[], outs=[],
                            sync_info=mybir.SyncInfo(on_wait=[w], on_update=[]),
                        )
                        nc.register_instruction(nop)
                        out.append(nop)
                    si.on_wait = waits[-max_waits:]
                    changed = True
                out.append(inst)
            if changed:
                bb.instructions = out


def _wrap_idx(idx_flat):
    """Pack a flat index list (len = multiple of 128) into the dma_gather idx
    tile layout: wrapped into 16 partitions, replicated to 8 Q7 cores."""
    n = len(idx_flat)
    w = np.zeros((16, n // 16), np.int16)
    q = np.arange(n)
    w[q % 16, q // 16] = idx_flat
    return np.tile(w, (8, 1))  # [128, n//16]


def _build_groups(S_t):
    """Pack consecutive dest tiles into groups with <= GMAX G-buffer slots
    (1 identity slot + S_t-1 edge slots per tile) and <= GMAX_T tiles."""
    groups = []
    ta = 0
    while ta < NTILES_D:
        # small first groups prime the gather->compute pipeline
        cap = 2 if ta == 0 else (4 if ta == 2 else GMAX_T)
        nt, slots = 0, 0
        while ta + nt < NTILES_D and nt < cap:
            add = int(S_t[ta + nt])  # 1 identity + (S_t-1) edges
            if slots + add > GMAX and nt > 0:
                break
            slots += add
            nt += 1
        groups.append((ta, ta + nt))
        ta += nt
    return groups


def _preprocess(x, L_cols, L_vals, EW_cols, EW_vals, NS_cols, NS_vals, coeffs, bias):
    """Host-side packing. Returns per-quarter gather metadata + shared consts."""
    cols_ops = [np.asarray(L_cols), np.asarray(EW_cols), np.asarray(NS_cols)]
    vals_ops = [np.asarray(L_vals), np.asarray(EW_vals), np.asarray(NS_vals)]

    # per-dest real edges (col < NVPREV) and pad-sums, over the 3 ops
    real_masks = [c < NVPREV for c in cols_ops]
    deg_ops = [m.sum(1) for m in real_masks]           # [NV] per op
    deg = sum(deg_ops)                                 # total real degree per dest
    s_pad = [np.where(~m, v, 0).sum(1).astype(np.float32)
             for m, v in zip(real_masks, vals_ops)]    # [NV] per op

    # permutation: degree-sorted (desc), padded dests at the end, dealt to quarters
    deg_pad = np.concatenate([deg, np.full(NPAD - NV, -1)])
    order = np.argsort(-deg_pad, kind="stable")
    pis = [order[q::NQ] for q in range(NQ)]            # [DPC] original dest ids

    # per-tile slot count, shared across quarters (program must be uniform)
    deg_of = lambda pi: np.where(pi < NV, deg_pad[np.minimum(pi, NV - 1)], 0).clip(0)
    S_t = np.zeros(NTILES_D, np.int64)
    for t in range(NTILES_D):
        m = 0
        for q in range(NQ):
            pi = pis[q][t * 128:(t + 1) * 128]
            m = max(m, int(deg_of(pi).max()))
        S_t[t] = 1 + m

    # pack per-quarter gather data
    quarters = []
    for q in range(NQ):
        pi = pis[q]
        idx0_cols, idx123_cols, vals_cols = [], [], []
        s5 = np.zeros((NTILES_D, 5, 128), np.float32)
        for t in range(NTILES_D):
            p_ids = pi[t * 128:(t + 1) * 128]
            real = p_ids < NV
            st = int(S_t[t])
            # identity slot
            id_idx = np.where(p_ids < NVPREV, p_ids, NVPREV).astype(np.int16)
            idx0_cols.append(_wrap_idx(np.where(real, id_idx, 0)))
            # edge slots
            eidx = np.zeros((st - 1, 128), np.int16)
            eval_ = np.zeros((128, st), np.float32)
            eval_[:, 0] = real.astype(np.float32)
            for p in range(128):
                n = p_ids[p]
                if n >= NV:
                    continue
                d = 0
                for k in range(3):
                    cs_, vs_ = cols_ops[k][n], vals_ops[k][n]
                    m = real_masks[k][n]
                    cc, vv = cs_[m], vs_[m]
                    eidx[d:d + len(cc), p] = (k * NVQ + cc).astype(np.int16)
                    eval_[p, 1 + d:1 + d + len(vv)] = vv
                    d += len(cc)
                s5[t, 1, p] = s_pad[0][n]
                s5[t, 2, p] = s_pad[1][n]
                s5[t, 3, p] = s_pad[2][n]
                s5[t, 4, p] = 1.0
            idx123_cols.append(_wrap_idx(eidx.reshape(-1)))
            vals_cols.append(eval_)
        quarters.append(dict(
            pi=pi,
            idx0=np.concatenate(idx0_cols, axis=1),      # [128, NTILES_D*8]
            idx123=np.concatenate(idx123_cols, axis=1),  # [128, wtot]
            vals=np.concatenate(vals_cols, axis=1),      # [128, stot]
            s5=s5.transpose(1, 0, 2).reshape(5, -1).astype(NP_BF16),
        ))

    # shared consts
    coeffs = np.asarray(coeffs, np.float32)
    callw = np.concatenate([coeffs[k] for k in range(4)], axis=1).astype(NP_BF16)
    csum = coeffs.sum(axis=1)                                      # [4, 64]
    cs = np.zeros((5, EW), np.float32)
    for k in range(1, 4):
        cs[k] = np.tile(csum[k], BPC)
    cs[4] = np.tile(np.asarray(bias, np.float32), BPC)
    cs = cs.astype(NP_BF16)
    eye = np.eye(128, dtype=NP_BF16)

    # xq per batch group: [BPC, 64, NVQ], col NVPREV = ones
    x = np.asarray(x, np.float32)
    xqs = []
    for g in range(NBG):
        xq = np.zeros((BPC, C, NVQ), np.float32)
        xq[:, :, :NVPREV] = x[g * BPC:(g + 1) * BPC]
        xq[:, :, NVPREV] = 1.0
        xqs.append(xq.astype(NP_BF16))

    return quarters, xqs, callw, cs, eye, S_t


def _build_program(S_t, wtot, stot, phase1=True, phase2=True,
                   n_queues=NQUEUES, single_packet=True):
    nc = bass.Bass(num_swdge_queues=n_queues,
                   dynamic_dma_scratch_size=SCRATCH)
    xq_ext = nc.declare_dram_parameter("xq", [BPC, C, NVQ], bf16, isOutput=False)
    callw_ext = nc.declare_dram_parameter("callw", [C, 4 * C], bf16, isOutput=False)
    cs_ext = nc.declare_dram_parameter("cs", [5, EW], bf16, isOutput=False)
    eye_ext = nc.declare_dram_parameter("eye", [128, 128], bf16, isOutput=False)
    idx0_ext = nc.declare_dram_parameter("idx0", [128, NTILES_D * 8], i16, isOutput=False)
    idx123_ext = nc.declare_dram_parameter("idx123", [128, wtot], i16, isOutput=False)
    vals_ext = nc.declare_dram_parameter("vals", [128, stot], f32, isOutput=False)
    s5_ext = nc.declare_dram_parameter("s5", [5, NTILES_D * 128], bf16, isOutput=False)
    out_ext = nc.declare_dram_parameter("out", [DPC, EW], bf16, isOutput=True)

    t0_dram = nc.dram_tensor("t0_scratch", [NVQ, EW], bf16)
    t123_dram = nc.dram_tensor("t123_scratch", [3 * NVQ, EW], bf16)

    groups = _build_groups(S_t)

    with tile.TileContext(nc) as tc:
        with (
            tc.tile_pool(name="const", bufs=1) as constp,
            tc.tile_pool(name="psum", bufs=4, space="PSUM") as psum,
        ):
            nc.gpsimd.load_library(library_config.mlp)
            callw_t = constp.tile([C, 4 * C], bf16)
            cs_t = constp.tile([5, EW], bf16)
            eye_t = constp.tile([128, 128], bf16)
            idx0_t = constp.tile([128, NTILES_D * 8], i16)
            idx123_t = constp.tile([128, wtot], i16)
            vals_t = constp.tile([128, stot], f32)
            s5_t = constp.tile([5, NTILES_D * 128], bf16)
            nc.sync.dma_start(callw_t[:], callw_ext[:])
            nc.sync.dma_start(cs_t[:], cs_ext[:])
            nc.sync.dma_start(eye_t[:], eye_ext[:])
            nc.sync.dma_start(idx0_t[:], idx0_ext[:])
            nc.sync.dma_start(idx123_t[:], idx123_ext[:])
            nc.sync.dma_start(vals_t[:], vals_ext[:])
            nc.sync.dma_start(s5_t[:], s5_ext[:])

            gq = [0]  # round-robin gather queue counter

            def _phase1():
                VTB = 8  # vertex tiles per table-write block
                with (
                    tc.tile_pool(name="xq", bufs=1) as xqp,
                    tc.tile_pool(name="zstage", bufs=3) as zst,
                ):
                    xq_t = [xqp.tile([C, NVQ], bf16, tag=f"xq{bb}", name=f"xq{bb}")
                            for bb in range(BPC)]
                    for bb in range(BPC):
                        nc.sync.dma_start(xq_t[bb][:], xq_ext[bb])
                    t123_3d = t123_dram.rearrange("(k n) e -> k n e", k=3)
                    for blk in range(0, NTILES_V, VTB):
                        nvt = min(VTB, NTILES_V - blk)
                        stall = zst.tile([128, VTB, 4, EW], bf16, tag="stall")
                        for vi in range(nvt):
                            vt = blk + vi
                            sl = slice(vt * 128, (vt + 1) * 128)
                            for bb in range(BPC):
                                ps = psum.tile([128, 4 * C], f32, tag="zps")
                                nc.tensor.matmul(ps[:], xq_t[bb][:, sl],
                                                 callw_t[:],
                                                 start=True, stop=True)
                                if (vi * BPC + bb) % 2 == 0:
                                    ceng = nc.scalar.copy
                                else:
                                    ceng = nc.vector.tensor_copy
                                ceng(
                                    stall[:, vi, :, bb * C:(bb + 1) * C],
                                    ps[:, :].rearrange("p (k c) -> p k c", k=4),
                                )
                        bsl = slice(blk * 128, (blk + nvt) * 128)
                        nc.sync.dma_start(
                            t0_dram[bsl].rearrange("(v p) e -> p v e", v=nvt),
                            stall[:, :nvt, 0, :])
                        for k in range(3):
                            nc.sync.dma_start(
                                t123_3d[k, bsl, :].rearrange(
                                    "(v p) e -> p v e", v=nvt),
                                stall[:, :nvt, 1 + k, :])

            reg_cache = {}

            def nreg(v):
                if v not in reg_cache:
                    reg_cache[v] = nc.gpsimd.to_reg(v)
                return reg_cache[v]

            def _gather(out_ap, tab, idxs, n):
                q = gq[0] % n_queues
                gq[0] += 1
                nc.gpsimd.dma_gather(out_ap, tab, idxs, num_idxs=n,
                                     num_idxs_reg=nreg(n), elem_size=EW,
                                     queue_num=q, single_packet=single_packet)

            def _phase2():
                OB = 3  # output tiles per DMA (81 = 27*3)
                with (
                    tc.tile_pool(name="work", bufs=2) as work,
                    tc.tile_pool(name="scaled", bufs=4) as scp,
                    tc.tile_pool(name="outp", bufs=3) as outp,
                ):
                    outt = [None]  # current output staging block
                    woff = 0   # global edge-slot offset (idx123 cols = 8/slot)
                    voff = 0   # global vals col offset
                    for (ta, tb) in groups:
                        nt = tb - ta
                        eslots = sum(int(S_t[u]) - 1 for u in range(ta, tb))
                        G = work.tile([128, GMAX, EW], bf16, tag="G")
                        if phase2 in (True, "gather", "compute"):
                            # identity rows for nt tiles in one call
                            _gather(G[:, 0:nt, :], t0_dram[:],
                                    idx0_t[:, ta * 8:tb * 8], nt * 128)
                            # edge rows: contiguous desc stream across tiles
                            e0 = 0
                            while e0 < eslots:
                                en = min(eslots - e0, CHUNK_SLOTS)
                                _gather(G[:, nt + e0:nt + e0 + en, :],
                                        t123_dram[:],
                                        idx123_t[:, (woff + e0) * 8:
                                                 (woff + e0 + en) * 8],
                                        en * 128)
                                e0 += en
                        eoff = nt
                        gv = voff
                        for ti in range(nt):
                            t = ta + ti
                            st = int(S_t[t])
                            acc = psum.tile([128, EW], f32, tag="acc")
                            nc.tensor.matmul(acc[:],
                                             s5_t[:, t * 128:(t + 1) * 128],
                                             cs_t[:], start=True, stop=False)
                            slots = [ti] + list(range(eoff, eoff + st - 1))
                            for j, scol in enumerate(slots):
                                sc = scp.tile([128, EW], bf16, tag="sc")
                                nc.vector.tensor_scalar_mul(
                                    sc[:], G[:, scol, :],
                                    vals_t[:, gv + j:gv + j + 1])
                                nc.tensor.matmul(acc[:], eye_t[:], sc[:],
                                                 start=False,
                                                 stop=(j == st - 1))
                            oi = t % OB
                            if oi == 0:
                                outt[0] = outp.tile([128, OB, EW], bf16,
                                                    tag="outt", name="outt")
                            nc.scalar.copy(outt[0][:, oi, :], acc[:])
                            if oi == OB - 1 or t == NTILES_D - 1:
                                ob = oi + 1
                                t0b = t - oi
                                nc.sync.dma_start(
                                    out_ext[t0b * 128:(t0b + ob) * 128]
                                    .rearrange("(v p) e -> p v e", v=ob),
                                    outt[0][:, :ob, :])
                            eoff += st - 1
                            gv += st
                        woff += eslots
                        voff = gv

            if phase1:
                _phase1()
            if phase2:
                _phase2()

    lower_extended_insts(nc)
    _fix_multiwait(nc)
    return nc


def kernel(x, L_cols, L_vals, EW_cols, EW_vals, NS_cols, NS_vals, coeffs, bias):
    quarters, xqs, callw, cs, eye, S_t = _preprocess(
        x, L_cols, L_vals, EW_cols, EW_vals, NS_cols, NS_vals, coeffs, bias)

    wtot = quarters[0]["idx123"].shape[1]
    stot = quarters[0]["vals"].shape[1]
    # quarters share S_t so widths match by construction
    assert all(qd["idx123"].shape[1] == wtot for qd in quarters)

    nc = _build_program(S_t, wtot, stot)

    in_maps = []
    for c in range(NCORES):
        g, q = divmod(c, NQ)
        qd = quarters[q]
        in_maps.append({
            "xq": xqs[g],
            "callw": callw,
            "cs": cs,
            "eye": eye,
            "idx0": qd["idx0"],
            "idx123": qd["idx123"],
            "vals": qd["vals"],
            "s5": qd["s5"],
        })

    res = run_bass_kernel_spmd(nc, in_maps, list(range(NCORES)))

    out = np.zeros((B, C, NV), np.float32)
    for c in range(NCORES):
        g, q = divmod(c, NQ)
        pi = quarters[q]["pi"]
        o = np.asarray(res.results[c]["out"]).astype(np.float32)  # [DPC, EW]
        valid = pi < NV
        rows = o[valid].reshape(-1, BPC, C)    # [nvalid, b, c]
        out[g * BPC:(g + 1) * BPC, :, pi[valid]] = rows.transpose(1, 2, 0)
    return out


# revision 15
# speedup vs baseline: 1.6550x; 1.0269x over previous
"""MeshConv-transpose Trainium2 kernel.

out[b,:,n] = (identity @ c0 + L_spmm @ c1 + EW_spmm @ c2 + NS_spmm @ c3 + bias)^T

Strategy (8 NeuronCores): cores = 2 batch-groups x 4 dest-quarters.
- Phase 1 (tables): T0 = [x;1] @ c0 and T123 = [x;1] @ c(1..3) on PE,
  rows packed [vertex, 4 batches x 64 ch] bf16 in HBM scratch.
- Phase 2 (sparse): per dest-tile (128 dests), gather identity + edge rows
  (dma_gather, 512B descriptors), scale each slot on DVE (tensor_scalar,
  4x bf16 mode), and accumulate slots on the PE via identity-matmuls into
  PSUM (out[m,n] = sum_k eye[k,m]*slot[k,n]); the per-dest pad-sum/bias
  const matmul (s5 @ cs) accumulates into the same PSUM bank. ACT copies
  PSUM -> bf16 out tile; host casts back to f32.
- Gathers are batched: tiles are packed into groups (<= GMAX slots); edge
  descriptors stream contiguously across tiles so one call covers many
  tiles. Identity descs batched per group. Big SWDGE ring, 2 queues.
- Dests are permuted (degree-sorted, round-robin dealt to quarters) so slot
  counts are uniform across cores; host un-permutes the output.
"""
import numpy as np

import concourse.bass as bass
import concourse.mybir as mybir
import concourse.tile as tile
from concourse import library_config
from concourse.bass_utils import run_bass_kernel_spmd
from concourse.library_overlay import lower_extended_insts

# ---- problem constants (hardcoded per harness contract) ----
NV = 40962
NVPREV = 10242
B = 8
C = 64
K_L = 7
K_G = 18

NBG = 2            # batch groups
BPC = B // NBG     # batches per core (4)
NQ = 4             # dest quarters
NCORES = NBG * NQ

NVQ = 10368        # xq rows per batch (=81*128): 10242 x-cols + ones col + pad
NTILES_V = NVQ // 128   # 81 z-build tiles
NPAD = 4 * NVQ     # padded dest count 41472
DPC = NVQ          # dests per core (10368)
NTILES_D = DPC // 128   # 81 dest tiles
EW = BPC * C       # elem width per table row (256 f32)

import os
GMAX = 96          # max G-buffer slots per tile group
GMAX_T = 8         # max tiles per group (identity gather <= 1024 descs)
CHUNK_SLOTS = int(os.environ.get("MC_CHUNK", "8"))    # edge-gather slots/call
SCRATCH = int(os.environ.get("MC_SCRATCH", "16384"))  # SWDGE ring bytes
NQUEUES = int(os.environ.get("MC_NQ", "2"))

f32 = mybir.dt.float32
bf16 = mybir.dt.bfloat16
i16 = mybir.dt.int16
NP_BF16 = mybir.dt.np(bf16)


def _fix_multiwait(nc, max_waits=1):
    """This walrus build accepts one sem-wait per instruction; hoist extras
    onto same-engine no-ops spliced before the instruction."""
    for f in nc.m.functions:
        for bb in f.blocks:
            out, changed = [], False
            for inst in bb.instructions:
                si = inst.sync_info
                waits = list(si.on_wait) if si and si.on_wait else []
                if len(waits) > max_waits:
                    for w in waits[:-max_waits]:
                        nop = mybir.InstNoOp(
                            name=nc.get_next_instruction_name(),
                            engine=inst.engine, ins=[], outs=[],
                            sync_info=mybir.SyncInfo(on_wait=[w], on_update=[]),
                        )
                        nc.register_instruction(nop)
                        out.append(nop)
                    si.on_wait = waits[-max_waits:]
                    changed = True
                out.append(inst)
            if changed:
                bb.instructions = out


def _wrap_idx(idx_flat):
    """Pack a flat index list (len = multiple of 128) into the dma_gather idx
    tile layout: wrapped into 16 partitions, replicated to 8 Q7 cores."""
    n = len(idx_flat)
    w = np.zeros((16, n // 16), np.int16)
    q = np.arange(n)
    w[q % 16, q // 16] = idx_flat
    return np.tile(w, (8, 1))  # [128, n//16]


def _build_groups(S_t):
    """Pack consecutive dest tiles into groups with <= GMAX G-buffer slots
    (1 identity slot + S_t-1 edge slots per tile) and <= GMAX_T tiles."""
    groups = []
    ta = 0
    while ta < NTILES_D:
        # small first groups prime the gather->compute pipeline
        cap = 2 if ta == 0 else (4 if ta == 2 else GMAX_T)
        nt, slots = 0, 0
        while ta + nt < NTILES_D and nt < cap:
            add = int(S_t[ta + nt])  # 1 identity + (S_t-1) edges
            if slots + add > GMAX and nt > 0:
                break
            slots += add
            nt += 1
        groups.append((ta, ta + nt))
        ta += nt
    return groups


def _preprocess(x, L_cols, L_vals, EW_cols, EW_vals, NS_cols, NS_vals, coeffs, bias):
    """Host-side packing. Returns per-quarter gather metadata + shared consts."""
    cols_ops = [np.asarray(L_cols), np.asarray(EW_cols), np.asarray(NS_cols)]
    vals_ops = [np.asarray(L_vals), np.asarray(EW_vals), np.asarray(NS_vals)]

    # per-dest real edges (col < NVPREV) and pad-sums, over the 3 ops
    real_masks = [c < NVPREV for c in cols_ops]
    deg_ops = [m.sum(1) for m in real_masks]           # [NV] per op
    deg = sum(deg_ops)                                 # total real degree per dest
    s_pad = [np.where(~m, v, 0).sum(1).astype(np.float32)
             for m, v in zip(real_masks, vals_ops)]    # [NV] per op

    # permutation: degree-sorted (desc), padded dests at the end, dealt to quarters
    deg_pad = np.concatenate([deg, np.full(NPAD - NV, -1)])
    order = np.argsort(-deg_pad, kind="stable")
    pis = [order[q::NQ] for q in range(NQ)]            # [DPC] original dest ids

    # per-tile slot count, shared across quarters (program must be uniform)
    deg_of = lambda pi: np.where(pi < NV, deg_pad[np.minimum(pi, NV - 1)], 0).clip(0)
    S_t = np.zeros(NTILES_D, np.int64)
    for t in range(NTILES_D):
        m = 0
        for q in range(NQ):
            pi = pis[q][t * 128:(t + 1) * 128]
            m = max(m, int(deg_of(pi).max()))
        S_t[t] = 1 + m

    # pack per-quarter gather data
    quarters = []
    for q in range(NQ):
        pi = pis[q]
        idx0_cols, idx123_cols, vals_cols = [], [], []
        s5 = np.zeros((NTILES_D, 5, 128), np.float32)
        for t in range(NTILES_D):
            p_ids = pi[t * 128:(t + 1) * 128]
            real = p_ids < NV
            st = int(S_t[t])
            # identity slot
            id_idx = np.where(p_ids < NVPREV, p_ids, NVPREV).astype(np.int16)
            idx0_cols.append(_wrap_idx(np.where(real, id_idx, 0)))
            # edge slots
            eidx = np.zeros((st - 1, 128), np.int16)
            eval_ = np.zeros((128, st), np.float32)
            eval_[:, 0] = real.astype(np.float32)
            for p in range(128):
                n = p_ids[p]
                if n >= NV:
                    continue
                d = 0
                for k in range(3):
                    cs_, vs_ = cols_ops[k][n], vals_ops[k][n]
                    m = real_masks[k][n]
                    cc, vv = cs_[m], vs_[m]
                    eidx[d:d + len(cc), p] = (k * NVQ + cc).astype(np.int16)
                    eval_[p, 1 + d:1 + d + len(vv)] = vv
                    d += len(cc)
                s5[t, 1, p] = s_pad[0][n]
                s5[t, 2, p] = s_pad[1][n]
                s5[t, 3, p] = s_pad[2][n]
                s5[t, 4, p] = 1.0
            idx123_cols.append(_wrap_idx(eidx.reshape(-1)))
            vals_cols.append(eval_)
        quarters.append(dict(
            pi=pi,
            idx0=np.concatenate(idx0_cols, axis=1),      # [128, NTILES_D*8]
            idx123=np.concatenate(idx123_cols, axis=1),  # [128, wtot]
            vals=np.concatenate(vals_cols, axis=1),      # [128, stot]
            s5=s5.transpose(1, 0, 2).reshape(5, -1).astype(NP_BF16),
        ))

    # shared consts
    coeffs = np.asarray(coeffs, np.float32)
    callw = np.concatenate([coeffs[k] for k in range(4)], axis=1).astype(NP_BF16)
    csum = coeffs.sum(axis=1)                                      # [4, 64]
    cs = np.zeros((5, EW), np.float32)
    for k in range(1, 4):
        cs[k] = np.tile(csum[k], BPC)
    cs[4] = np.tile(np.asarray(bias, np.float32), BPC)
    cs = cs.astype(NP_BF16)
    eye = np.eye(128, dtype=NP_BF16)

    # xq per batch group: [BPC, 64, NVQ], col NVPREV = ones
    x = np.asarray(x, np.float32)
    xqs = []
    for g in range(NBG):
        xq = np.zeros((BPC, C, NVQ), np.float32)
        xq[:, :, :NVPREV] = x[g * BPC:(g + 1) * BPC]
        xq[:, :, NVPREV] = 1.0
        xqs.append(xq.astype(NP_BF16))

    return quarters, xqs, callw, cs, eye, S_t


def _build_program(S_t, wtot, stot, phase1=True, phase2=True,
                   n_queues=NQUEUES, single_packet=True):
    nc = bass.Bass(num_swdge_queues=n_queues,
                   dynamic_dma_scratch_size=SCRATCH)
    xq_ext = nc.declare_dram_parameter("xq", [BPC, C, NVQ], bf16, isOutput=False)
    callw_ext = nc.declare_dram_parameter("callw", [C, 4 * C], bf16, isOutput=False)
    cs_ext = nc.declare_dram_parameter("cs", [5, EW], bf16, isOutput=False)
    eye_ext = nc.declare_dram_parameter("eye", [128, 128], bf16, isOutput=False)
    idx0_ext = nc.declare_dram_parameter("idx0", [128, NTILES_D * 8], i16, isOutput=False)
    idx123_ext = nc.declare_dram_parameter("idx123", [128, wtot], i16, isOutput=False)
    vals_ext = nc.declare_dram_parameter("vals", [128, stot], f32, isOutput=False)
    s5_ext = nc.declare_dram_parameter("s5", [5, NTILES_D * 128], bf16, isOutput=False)
    out_ext = nc.declare_dram_parameter("out", [DPC, EW], bf16, isOutput=True)

    t0_dram = nc.dram_tensor("t0_scratch", [NVQ, EW], bf16)
    t123_dram = nc.dram_tensor("t123_scratch", [3 * NVQ, EW], bf16)

    groups = _build_groups(S_t)

    with tile.TileContext(nc) as tc:
        with (
            tc.tile_pool(name="const", bufs=1) as constp,
            tc.tile_pool(name="psum", bufs=4, space="PSUM") as psum,
        ):
            nc.gpsimd.load_library(library_config.mlp)
            callw_t = constp.tile([C, 4 * C], bf16)
            cs_t = constp.tile([5, EW], bf16)
            eye_t = constp.tile([128, 128], bf16)
            idx0_t = constp.tile([128, NTILES_D * 8], i16)
            idx123_t = constp.tile([128, wtot], i16)
            vals_t = constp.tile([128, stot], f32)
            s5_t = constp.tile([5, NTILES_D * 128], bf16)
            nc.sync.dma_start(callw_t[:], callw_ext[:])

            def _load_consts():
                # off the phase1 critical path; issued after the xq loads
                nc.sync.dma_start(cs_t[:], cs_ext[:])
                nc.sync.dma_start(eye_t[:], eye_ext[:])
                nc.sync.dma_start(idx0_t[:], idx0_ext[:])
                nc.sync.dma_start(idx123_t[:], idx123_ext[:])
                nc.sync.dma_start(vals_t[:], vals_ext[:])
                nc.sync.dma_start(s5_t[:], s5_ext[:])

            gq = [0]  # round-robin gather queue counter

            def _phase1():
                VTB = 8  # vertex tiles per table-write block
                with (
                    tc.tile_pool(name="xq", bufs=1) as xqp,
                    tc.tile_pool(name="zstage", bufs=3) as zst,
                ):
                    xq_t = [xqp.tile([C, NVQ], bf16, tag=f"xq{bb}", name=f"xq{bb}")
                            for bb in range(BPC)]
                    for bb in range(BPC):
                        nc.sync.dma_start(xq_t[bb][:], xq_ext[bb])
                    _load_consts()
                    t123_3d = t123_dram.rearrange("(k n) e -> k n e", k=3)
                    for blk in range(0, NTILES_V, VTB):
                        nvt = min(VTB, NTILES_V - blk)
                        stall = zst.tile([128, VTB, 4, EW], bf16, tag="stall")
                        for vi in range(nvt):
                            vt = blk + vi
                            sl = slice(vt * 128, (vt + 1) * 128)
                            for bp in range(BPC // 2):
                                # two batches share one PSUM bank, one copy
                                ps = psum.tile([128, 8 * C], f32, tag="zps")
                                for h in range(2):
                                    bb = 2 * bp + h
                                    nc.tensor.matmul(
                                        ps[:, h * 4 * C:(h + 1) * 4 * C],
                                        xq_t[bb][:, sl], callw_t[:],
                                        start=True, stop=True)
                                if (vi * 2 + bp) % 2 == 0:
                                    ceng = nc.scalar.copy
                                else:
                                    ceng = nc.vector.tensor_copy
                                ceng(
                                    stall[:, vi, :, 2 * bp * C:
                                          (2 * bp + 2) * C]
                                    .rearrange("p k (b c) -> p k b c", b=2),
                                    ps[:, :].rearrange(
                                        "p (b k c) -> p k b c", b=2, k=4),
                                )
                        bsl = slice(blk * 128, (blk + nvt) * 128)
                        nc.sync.dma_start(
                            t0_dram[bsl].rearrange("(v p) e -> p v e", v=nvt),
                            stall[:, :nvt, 0, :])
                        for k in range(3):
                            nc.sync.dma_start(
                                t123_3d[k, bsl, :].rearrange(
                                    "(v p) e -> p v e", v=nvt),
                                stall[:, :nvt, 1 + k, :])

            reg_cache = {}

            def nreg(v):
                if v not in reg_cache:
                    reg_cache[v] = nc.gpsimd.to_reg(v)
                return reg_cache[v]

            def _gather(out_ap, tab, idxs, n):
                q = gq[0] % n_queues
                gq[0] += 1
                nc.gpsimd.dma_gather(out_ap, tab, idxs, num_idxs=n,
                                     num_idxs_reg=nreg(n), elem_size=EW,
                                     queue_num=q, single_packet=single_packet)

            def _phase2():
                OB = 3  # output tiles per DMA (81 = 27*3)
                with (
                    tc.tile_pool(name="work", bufs=2) as work,
                    tc.tile_pool(name="scaled", bufs=4) as scp,
                    tc.tile_pool(name="outp", bufs=3) as outp,
                ):
                    outt = [None]  # current output staging block
                    woff = 0   # global edge-slot offset (idx123 cols = 8/slot)
                    voff = 0   # global vals col offset
                    for (ta, tb) in groups:
                        nt = tb - ta
                        eslots = sum(int(S_t[u]) - 1 for u in range(ta, tb))
                        G = work.tile([128, GMAX, EW], bf16, tag="G")
                        if phase2 in (True, "gather", "compute"):
                            # identity rows for nt tiles in one call
                            _gather(G[:, 0:nt, :], t0_dram[:],
                                    idx0_t[:, ta * 8:tb * 8], nt * 128)
                            # edge rows: contiguous desc stream across tiles
                            e0 = 0
                            while e0 < eslots:
                                en = min(eslots - e0, CHUNK_SLOTS)
                                _gather(G[:, nt + e0:nt + e0 + en, :],
                                        t123_dram[:],
                                        idx123_t[:, (woff + e0) * 8:
                                                 (woff + e0 + en) * 8],
                                        en * 128)
                                e0 += en
                        eoff = nt
                        gv = voff
                        for ti in range(nt):
                            t = ta + ti
                            st = int(S_t[t])
                            acc = psum.tile([128, EW], f32, tag="acc")
                            nc.tensor.matmul(acc[:],
                                             s5_t[:, t * 128:(t + 1) * 128],
                                             cs_t[:], start=True, stop=False)
                            slots = [ti] + list(range(eoff, eoff + st - 1))
                            for j, scol in enumerate(slots):
                                sc = scp.tile([128, EW], bf16, tag="sc")
                                nc.vector.tensor_scalar_mul(
                                    sc[:], G[:, scol, :],
                                    vals_t[:, gv + j:gv + j + 1])
                                nc.tensor.matmul(acc[:], eye_t[:], sc[:],
                                                 start=False,
                                                 stop=(j == st - 1))
                            oi = t % OB
                            if oi == 0:
                                outt[0] = outp.tile([128, OB, EW], bf16,
                                                    tag="outt", name="outt")
                            nc.scalar.copy(outt[0][:, oi, :], acc[:])
                            if oi == OB - 1 or t == NTILES_D - 1:
                                ob = oi + 1
                                t0b = t - oi
                                nc.sync.dma_start(
                                    out_ext[t0b * 128:(t0b + ob) * 128]
                                    .rearrange("(v p) e -> p v e", v=ob),
                                    outt[0][:, :ob, :])
                            eoff += st - 1
                            gv += st
                        woff += eslots
                        voff = gv

            if phase1:
                _phase1()
            else:
                _load_consts()
            if phase2:
                _phase2()

    lower_extended_insts(nc)
    _fix_multiwait(nc)
    return nc


def kernel(x, L_cols, L_vals, EW_cols, EW_vals, NS_cols, NS_vals, coeffs, bias):
    quarters, xqs, callw, cs, eye, S_t = _preprocess(
        x, L_cols, L_vals, EW_cols, EW_vals, NS_cols, NS_vals, coeffs, bias)

    wtot = quarters[0]["idx123"].shape[1]
    stot = quarters[0]["vals"].shape[1]
    # quarters share S_t so widths match by construction
    assert all(qd["idx123"].shape[1] == wtot for qd in quarters)

    nc = _build_program(S_t, wtot, stot)

    in_maps = []
    for c in range(NCORES):
        g, q = divmod(c, NQ)
        qd = quarters[q]
        in_maps.append({
            "xq": xqs[g],
            "callw": callw,
            "cs": cs,
            "eye": eye,
            "idx0": qd["idx0"],
            "idx123": qd["idx123"],
            "vals": qd["vals"],
            "s5": qd["s5"],
        })

    res = run_bass_kernel_spmd(nc, in_maps, list(range(NCORES)))

    out = np.zeros((B, C, NV), np.float32)
    for c in range(NCORES):
        g, q = divmod(c, NQ)
        pi = quarters[q]["pi"]
        o = np.asarray(res.results[c]["out"]).astype(np.float32)  # [DPC, EW]
        valid = pi < NV
        rows = o[valid].reshape(-1, BPC, C)    # [nvalid, b, c]
        out[g * BPC:(g + 1) * BPC, :, pi[valid]] = rows.transpose(1, 2, 0)
    return out
